# revision 1
# baseline (speedup 1.0000x reference)
"""BNNLinear sampling kernel for Trainium2, data-parallel over 8 NeuronCores.

Computes h[m,c] = sum_r x_ext[m,r] * (mu[c,r] + sqrt(var[c,r]) * E[m,c,r])
with x_ext = concat([x, ones], axis=1), for
  x  [256, 512] f32, mu/var [512, 513] f32, E [256, 512, 513] f32.

Strategy (memory-bound; E is ~269 MB and must stream through HBM once):
 - Shard the sample axis m across the 8 cores (32 samples each).
 - Host-side LAYOUT ONLY: per-sample transpose of E to [r, c] blocked as
   [m, p, k, c] (r = 128k + p) so each per-sample DMA is one contiguous 1 MB
   transfer landing as SBUF tile [128p, 4k, 512c]; mu/var/x are pre-transposed
   the same way (tiny). All arithmetic (sqrt, multiplies, reductions) is
   on-chip.
 - The E stream is split across BOTH HWDGE queues (SP: even samples plus
   the output blocks, Act: odd samples plus the constants): a single HWDGE
   queue tops out ~286 GB/s on this hardware while two queues together
   sustain ~380-430 GB/s. The first samples issue before anything else so
   the stream starts at t~0 and the DMA engines never idle.
 - Per sample: one DVE tensor_tensor B = E_t * sqrt(var)_t ([128, 2048],
   output rounded to f32r), then 5 f32r PE matmuls into a private [1, 512]
   PSUM row: a preload matmul (stationary = identity column, moving = hbs
   [32, 512], start=True) seeds the row with the mean/bias term
   hbs[m,c] = x@mu^T + mu_bias + sqrt(var_bias)*E_bias, then 4 matmuls
   (stationary = x column chunk [128, 1]) accumulate sum_r over the 4
   r-chunks on top.  f32r streams the 512-wide moving operand at 1
   cycle/row (vs 4 for plain f32), keeping the PE far below the DMA
   stream; f32r requires dst partition 0, hence one PSUM row per bank.
 - Each finished PSUM row is drained by an Act copy (DMA cannot read PSUM;
   engine APs must start at partition 0) into [1, 4, C] staging blocks that
   are DMA'd to the DRAM output shard 4 rows at a time.
"""

import numpy as np
from contextlib import ExitStack

import concourse.bacc as bacc
import concourse.mybir as mybir
import concourse.tile as tile
from concourse.bass_utils import run_bass_kernel_spmd

F32 = mybir.dt.float32
F32R = mybir.dt.float32r  # PE fast-fp32 mode: 1 cycle/row (vs 4 for fp32)
                          # when the moving free dim >= 256; same 4-byte data.

N_CORES = 8
M_TOTAL = 256
M_SH = M_TOTAL // N_CORES  # 32 samples per core
C = 512
R_IN = 512                 # r chunks: 4 x 128
KCH = 4

_COMPILED = None


def _build_program(repeat=1):
    nc = bacc.Bacc("TRN2", target_bir_lowering=False, debug=False)

    et_d = nc.dram_tensor("et", [M_SH, 128, KCH, C], F32, kind="ExternalInput").ap()
    eb_d = nc.dram_tensor("eb", [M_SH, C], F32, kind="ExternalInput").ap()
    xt_d = nc.dram_tensor("xt", [128, KCH, M_SH], F32R, kind="ExternalInput").ap()
    mu_d = nc.dram_tensor("mu_t", [128, KCH, C], F32R, kind="ExternalInput").ap()
    mub_d = nc.dram_tensor("mu_b", [1, C], F32, kind="ExternalInput").ap()
    var_d = nc.dram_tensor("var_t", [128, KCH, C], F32, kind="ExternalInput").ap()
    varb_d = nc.dram_tensor("var_b", [1, C], F32, kind="ExternalInput").ap()
    id32_d = nc.dram_tensor("id32", [M_SH, M_SH], F32R, kind="ExternalInput").ap()
    out_d = nc.dram_tensor("out", [M_SH, C], F32, kind="ExternalOutput").ap()

    with tile.TileContext(nc) as tc, ExitStack() as ctx:
        const = ctx.enter_context(tc.tile_pool(name="const", bufs=1))
        work = ctx.enter_context(tc.tile_pool(name="work", bufs=8))
        bpool = ctx.enter_context(tc.tile_pool(name="bpool", bufs=6))
        spool = ctx.enter_context(tc.tile_pool(name="spool", bufs=4))
        psum = ctx.enter_context(tc.tile_pool(name="psum", bufs=6, space="PSUM"))
        psum1 = ctx.enter_context(tc.tile_pool(name="psum1", bufs=1, space="PSUM"))

        # ---- E stream: issue the first sample loads before anything else so
        # the SP queue starts the bulk stream at t~0; the work pool's bufs
        # keep it rolling ahead of compute for the rest of the loop.
        # Steady-state E loads alternate between the SP and Act HWDGE
        # queues: a single queue tops out ~286 GB/s on HW, two sustain ~382.
        n_pre = 6
        pre_tiles = []
        for m in range(n_pre):
            e_t = work.tile([128, KCH, C], F32, tag="et")
            nc.sync.dma_start(e_t[:], et_d[m])
            pre_tiles.append(e_t)

        # ---- constants, all on the Act HWDGE queue (var first: sqrt path)
        var_sb = const.tile([128, KCH, C], F32)
        nc.scalar.dma_start(var_sb[:], var_d)
        xt_sb = const.tile([128, KCH, M_SH], F32R)
        nc.scalar.dma_start(xt_sb[:], xt_d)
        id32_sb = const.tile([M_SH, M_SH], F32R)
        nc.scalar.dma_start(id32_sb[:], id32_d)
        varb_sb = const.tile([1, C], F32)
        nc.scalar.dma_start(varb_sb[:], varb_d)
        mu_sb = const.tile([128, KCH, C], F32R)
        nc.scalar.dma_start(mu_sb[:], mu_d)
        mub_sb = const.tile([1, C], F32)
        nc.scalar.dma_start(mub_sb[:], mub_d)
        eb_sb = const.tile([M_SH, C], F32)
        nc.scalar.dma_start(eb_sb[:], eb_d)

        s_sb = const.tile([128, KCH, C], F32)
        nc.scalar.sqrt(s_sb[:], var_sb[:])
        sb_sb = const.tile([1, C], F32)
        nc.scalar.sqrt(sb_sb[:], varb_sb[:])

        ones32 = const.tile([1, M_SH], F32)
        nc.vector.memset(ones32[:], 1.0)

        # broadcast sqrt(var) bias row to 32 partitions via PE outer product
        ps_b = psum1.tile([M_SH, C], F32)
        nc.tensor.matmul(ps_b[:], lhsT=ones32[:], rhs=sb_sb[:], start=True, stop=True)
        sbb_sb = const.tile([M_SH, C], F32)
        nc.scalar.copy(sbb_sb[:], ps_b[:])

        # h1 = x_t @ mu_t + mu bias row  -> [32, 512] psum, rows = samples
        h1_ps = psum1.tile([M_SH, C], F32)
        for k in range(KCH):
            nc.tensor.matmul(
                h1_ps[:],
                lhsT=xt_sb[:, k, :],
                rhs=mu_sb[:, k, :],
                start=(k == 0), stop=False,
            )
        nc.tensor.matmul(h1_ps[:], lhsT=ones32[:], rhs=mub_sb[:], start=False, stop=True)

        # hbs[m, c] = h1[m, c] + Eb[m, c] * sqrt(var)[c, 512]   (stored f32r:
        # it re-enters the PE as the moving operand of the preload matmul)
        ebs_sb = const.tile([M_SH, C], F32)
        nc.vector.tensor_tensor(
            out=ebs_sb[:], in0=eb_sb[:], in1=sbb_sb[:], op=mybir.AluOpType.mult
        )
        hbs_sb = const.tile([M_SH, C], F32R)
        nc.vector.tensor_tensor(
            out=hbs_sb[:], in0=h1_ps[:], in1=ebs_sb[:], op=mybir.AluOpType.add
        )

        # ---- main loop over samples ----
        for r_i in range(repeat):
            for m in range(M_SH):
                if r_i == 0 and m < n_pre:
                    e_t = pre_tiles[m]
                else:
                    e_t = work.tile([128, KCH, C], F32, tag="et")
                    (nc.sync if m % 2 == 0 else nc.scalar).dma_start(e_t[:], et_d[m])
                bt = bpool.tile([128, KCH, C], F32R, tag="bt")
                if m == M_SH - 1:
                    # last sample of the round: chunk the multiply so each
                    # matmul can start as soon as its r-chunk is scaled,
                    # shortening the pipeline drain
                    for k in range(KCH):
                        nc.vector.tensor_tensor(
                            out=bt[:, k, :], in0=e_t[:, k, :], in1=s_sb[:, k, :],
                            op=mybir.AluOpType.mult,
                        )
                else:
                    nc.vector.tensor_tensor(
                        out=bt[:], in0=e_t[:], in1=s_sb[:], op=mybir.AluOpType.mult
                    )
                pm = psum.tile([1, C], F32, tag="pm")
                nc.tensor.matmul(
                    pm[:], lhsT=id32_sb[:, m : m + 1], rhs=hbs_sb[:],
                    start=True, stop=False, skip_group_check=True,
                )
                for k in range(KCH):
                    nc.tensor.matmul(
                        pm[:],
                        lhsT=xt_sb[:, k, m : m + 1],
                        rhs=bt[:, k, :],
                        start=False,
                        stop=(k == KCH - 1),
                        skip_group_check=True,
                    )
                # drain: Act copy (engine APs must stay at partition 0;
                # DMA can't read PSUM) into a [1, 4, C] staging block,
                # DMA'd out (Act queue) once 4 rows are in
                if m % 4 == 0:
                    st = spool.tile([1, 4, C], F32, tag="st")
                nc.scalar.copy(st[:, m % 4, :], pm[:])
                if m % 4 == 3:
                    nc.sync.dma_start(out_d[m - 3 : m + 1, :], st[:, :, :])

    nc.compile()
    return nc


def _prep_inputs(x, mu, var, E):
    x = np.ascontiguousarray(x, dtype=np.float32)
    mu = np.ascontiguousarray(mu, dtype=np.float32)
    var = np.ascontiguousarray(var, dtype=np.float32)
    E = np.ascontiguousarray(E, dtype=np.float32)

    # mu/var transposed-blocked: [p, k, c] with r = 128k + p (r < 512)
    def blk(t):
        tt = np.ascontiguousarray(t.T[:R_IN])          # [512, 512] (r, c)
        return np.ascontiguousarray(
            tt.reshape(KCH, 128, C).transpose(1, 0, 2)  # [128, 4, 512]
        )

    mu_t = blk(mu)
    var_t = blk(var)
    mu_b = np.ascontiguousarray(mu[:, R_IN]).reshape(1, C)
    var_b = np.ascontiguousarray(var[:, R_IN]).reshape(1, C)
    id32 = np.eye(M_SH, dtype=np.float32)

    # E per-sample transpose + block: [m, p, k, c], r = 128k + p
    et = np.ascontiguousarray(
        E.transpose(0, 2, 1)[:, :R_IN, :]              # [256, 512(r), 512(c)]
        .reshape(M_TOTAL, KCH, 128, C)
        .transpose(0, 2, 1, 3)                          # [256, 128, 4, 512]
    )
    eb = np.ascontiguousarray(E[:, :, R_IN])            # [256, 512]

    # x transposed-blocked per core: [p, k, m_local]
    in_maps = []
    for core in range(N_CORES):
        sl = slice(core * M_SH, (core + 1) * M_SH)
        xs = x[sl]                                      # [32, 512]
        xt = np.ascontiguousarray(
            xs.T.reshape(KCH, 128, M_SH).transpose(1, 0, 2)  # [128, 4, 32]
        )
        in_maps.append({
            "et": np.ascontiguousarray(et[sl]),
            "eb": np.ascontiguousarray(eb[sl]),
            "xt": xt,
            "mu_t": mu_t,
            "var_t": var_t,
            "mu_b": mu_b,
            "var_b": var_b,
            "id32": id32,
        })
    return in_maps


def kernel(x, mu, var, E, shape=None, _trace=False, **_ignored):
    global _COMPILED
    if _COMPILED is None:
        _COMPILED = _build_program()
    nc = _COMPILED
    in_maps = _prep_inputs(np.asarray(x), np.asarray(mu), np.asarray(var), np.asarray(E))
    res = run_bass_kernel_spmd(
        nc, in_maps, core_ids=list(range(N_CORES)), trace=_trace,
    )
    out = np.concatenate([res.results[i]["out"] for i in range(N_CORES)], axis=0)
    if _trace:
        kernel._last_results = res
    return out



# revision 2
# speedup vs baseline: 1.5809x; 1.5809x over previous
"""BNNLinear sampling kernel for Trainium2, data-parallel over 8 NeuronCores.

Computes h[m,c] = sum_r x_ext[m,r] * (mu[c,r] + sqrt(var[c,r]) * E[m,c,r])
with x_ext = concat([x, ones], axis=1), for
  x  [256, 512] f32, mu/var [512, 513] f32, E [256, 512, 513] f32.

Strategy (memory-bound; E must stream through HBM once per pass):
 - Shard the sample axis m across the 8 cores (32 samples each).
 - Host-side layout + fp16 downcast: per-sample transpose of E to [r, c]
   blocked as [m, p, k, c] (r = 128k + p) so each per-sample DMA is one
   contiguous 512 KB fp16 transfer landing as SBUF tile [128p, 4k, 512c];
   mu/var/x are pre-transposed the same way (tiny). fp16 halves the HBM
   stream against the ~358 GB/s per-core HBM roofline, and the rel-err
   contribution (~sqrt(513)*2^-11 of the term scale) is ~100x below the
   2e-2 gate. All arithmetic (sqrt, multiplies, reductions) is on-chip;
   accumulation stays f32 in PSUM.
 - The E stream is split across BOTH HWDGE queues (SP: even samples plus
   sample 31 and the output blocks; Act: odd samples plus the constants,
   sized so both rings carry ~8.9 MB): a single HWDGE queue tops out
   ~286 GB/s while two queues together sustain ~380 GB/s.
 - Per sample: one DVE tensor_tensor B = E_t * sqrt(var)_t ([128, 2048]
   fp16 in/out -> 2x perf mode, ~1.1 us), then 5 PE matmuls into a private
   [1, 512] PSUM row: a preload matmul (stationary = identity column,
   moving = hbs [32, 512] f32r, start=True) seeds the row with the
   mean/bias term hbs[m,c] = x@mu^T + mu_bias + sqrt(var_bias)*E_bias,
   then 4 fp16 matmuls (stationary = x column chunk [128, 1]) accumulate
   sum_r over the 4 r-chunks on top.
 - Each finished PSUM row is drained by an Act copy (DMA cannot read PSUM;
   engine APs must start at partition 0) into [1, 4, C] staging blocks that
   are DMA'd to the DRAM output shard 4 rows at a time.
"""

import numpy as np
from contextlib import ExitStack

import concourse.bacc as bacc
import concourse.mybir as mybir
import concourse.tile as tile
from concourse.bass_utils import run_bass_kernel_spmd

F32 = mybir.dt.float32
F32R = mybir.dt.float32r  # PE fast-fp32 mode: 1 cycle/row when free dim >= 256
F16 = mybir.dt.float16

N_CORES = 8
M_TOTAL = 256
M_SH = M_TOTAL // N_CORES  # 32 samples per core
C = 512
R_IN = 512                 # r chunks: 4 x 128
KCH = 4

_COMPILED = None


def _equeue(nc, m):
    # queue balance: SP carries evens + sample 31 (+ output blocks),
    # Act carries odds + the ~1.06 MB of constants -> ~8.9 MB per ring
    return nc.sync if (m % 2 == 0 or m == M_SH - 1) else nc.scalar


def _build_program(repeat=1):
    nc = bacc.Bacc("TRN2", target_bir_lowering=False, debug=False)

    et_d = nc.dram_tensor("et", [M_SH, 128, KCH, C], F16, kind="ExternalInput").ap()
    eb_d = nc.dram_tensor("eb", [M_SH, C], F32, kind="ExternalInput").ap()
    xt_d = nc.dram_tensor("xt", [128, KCH, M_SH], F16, kind="ExternalInput").ap()
    mu_d = nc.dram_tensor("mu_t", [128, KCH, C], F16, kind="ExternalInput").ap()
    mub_d = nc.dram_tensor("mu_b", [1, C], F32, kind="ExternalInput").ap()
    var_d = nc.dram_tensor("var_t", [128, KCH, C], F16, kind="ExternalInput").ap()
    varb_d = nc.dram_tensor("var_b", [1, C], F32, kind="ExternalInput").ap()
    id32_d = nc.dram_tensor("id32", [M_SH, M_SH], F32R, kind="ExternalInput").ap()
    out_d = nc.dram_tensor("out", [M_SH, C], F32, kind="ExternalOutput").ap()

    with tile.TileContext(nc) as tc, ExitStack() as ctx:
        const = ctx.enter_context(tc.tile_pool(name="const", bufs=1))
        work = ctx.enter_context(tc.tile_pool(name="work", bufs=10))
        bpool = ctx.enter_context(tc.tile_pool(name="bpool", bufs=6))
        spool = ctx.enter_context(tc.tile_pool(name="spool", bufs=4))
        psum = ctx.enter_context(tc.tile_pool(name="psum", bufs=6, space="PSUM"))
        psum1 = ctx.enter_context(tc.tile_pool(name="psum1", bufs=1, space="PSUM"))

        # ---- E stream: issue the first sample loads before anything else so
        # the SP queue starts the bulk stream at t~0; the work pool's bufs
        # keep it rolling ahead of compute for the rest of the loop.
        n_pre = 6
        pre_tiles = []
        for m in range(n_pre):
            e_t = work.tile([128, KCH, C], F16, tag="et")
            nc.sync.dma_start(e_t[:], et_d[m])
            pre_tiles.append(e_t)

        # ---- constants, all on the Act HWDGE queue (var first: sqrt path)
        var_sb = const.tile([128, KCH, C], F16)
        nc.scalar.dma_start(var_sb[:], var_d)
        xt_sb = const.tile([128, KCH, M_SH], F16)
        nc.scalar.dma_start(xt_sb[:], xt_d)
        id32_sb = const.tile([M_SH, M_SH], F32R)
        nc.scalar.dma_start(id32_sb[:], id32_d)
        varb_sb = const.tile([1, C], F32)
        nc.scalar.dma_start(varb_sb[:], varb_d)
        mu_sb = const.tile([128, KCH, C], F16)
        nc.scalar.dma_start(mu_sb[:], mu_d)
        mub_sb = const.tile([1, C], F32)
        nc.scalar.dma_start(mub_sb[:], mub_d)
        eb_sb = const.tile([M_SH, C], F32)
        nc.scalar.dma_start(eb_sb[:], eb_d)

        s_sb = const.tile([128, KCH, C], F16)
        nc.scalar.sqrt(s_sb[:], var_sb[:])
        sb_sb = const.tile([1, C], F32)
        nc.scalar.sqrt(sb_sb[:], varb_sb[:])

        ones32 = const.tile([1, M_SH], F32)
        nc.vector.memset(ones32[:], 1.0)

        # broadcast sqrt(var) bias row to 32 partitions via PE outer product
        ps_b = psum1.tile([M_SH, C], F32)
        nc.tensor.matmul(ps_b[:], lhsT=ones32[:], rhs=sb_sb[:], start=True, stop=True)
        sbb_sb = const.tile([M_SH, C], F32)
        nc.scalar.copy(sbb_sb[:], ps_b[:])

        # h1 = x_t @ mu_t + mu bias row  -> [32, 512] psum, rows = samples
        h1_ps = psum1.tile([M_SH, C], F32)
        for k in range(KCH):
            nc.tensor.matmul(
                h1_ps[:],
                lhsT=xt_sb[:, k, :],
                rhs=mu_sb[:, k, :],
                start=(k == 0), stop=False,
            )
        nc.tensor.matmul(h1_ps[:], lhsT=ones32[:], rhs=mub_sb[:], start=False, stop=True)

        # hbs[m, c] = h1[m, c] + Eb[m, c] * sqrt(var)[c, 512]   (stored f32r:
        # it re-enters the PE as the moving operand of the preload matmul)
        ebs_sb = const.tile([M_SH, C], F32)
        nc.vector.tensor_tensor(
            out=ebs_sb[:], in0=eb_sb[:], in1=sbb_sb[:], op=mybir.AluOpType.mult
        )
        hbs_sb = const.tile([M_SH, C], F32R)
        nc.vector.tensor_tensor(
            out=hbs_sb[:], in0=h1_ps[:], in1=ebs_sb[:], op=mybir.AluOpType.add
        )

        # ---- main loop over samples ----
        for r_i in range(repeat):
            for m in range(M_SH):
                if r_i == 0 and m < n_pre:
                    e_t = pre_tiles[m]
                else:
                    e_t = work.tile([128, KCH, C], F16, tag="et")
                    _equeue(nc, m).dma_start(e_t[:], et_d[m])
                bt = bpool.tile([128, KCH, C], F16, tag="bt")
                if m == M_SH - 1:
                    # last sample of the round: chunk the multiply so each
                    # matmul can start as soon as its r-chunk is scaled,
                    # shortening the pipeline drain
                    for k in range(KCH):
                        nc.vector.tensor_tensor(
                            out=bt[:, k, :], in0=e_t[:, k, :], in1=s_sb[:, k, :],
                            op=mybir.AluOpType.mult,
                        )
                else:
                    nc.vector.tensor_tensor(
                        out=bt[:], in0=e_t[:], in1=s_sb[:], op=mybir.AluOpType.mult
                    )
                pm = psum.tile([1, C], F32, tag="pm")
                nc.tensor.matmul(
                    pm[:], lhsT=id32_sb[:, m : m + 1], rhs=hbs_sb[:],
                    start=True, stop=False, skip_group_check=True,
                )
                for k in range(KCH):
                    nc.tensor.matmul(
                        pm[:],
                        lhsT=xt_sb[:, k, m : m + 1],
                        rhs=bt[:, k, :],
                        start=False,
                        stop=(k == KCH - 1),
                        skip_group_check=True,
                    )
                # drain: Act copy (engine APs must stay at partition 0;
                # DMA can't read PSUM) into a [1, 4, C] staging block,
                # DMA'd out (SP queue) once 4 rows are in
                if m % 4 == 0:
                    st = spool.tile([1, 4, C], F32, tag="st")
                nc.scalar.copy(st[:, m % 4, :], pm[:])
                if m % 4 == 3:
                    nc.sync.dma_start(out_d[m - 3 : m + 1, :], st[:, :, :])

    nc.compile()
    return nc


def _prep_inputs(x, mu, var, E):
    x = np.asarray(x, dtype=np.float32)
    mu = np.asarray(mu, dtype=np.float32)
    var = np.asarray(var, dtype=np.float32)
    E = np.asarray(E)

    # mu/var transposed-blocked: [p, k, c] with r = 128k + p (r < 512)
    def blk(t):
        tt = np.ascontiguousarray(t.T[:R_IN], dtype=np.float16)  # [512, 512] (r, c)
        return np.ascontiguousarray(
            tt.reshape(KCH, 128, C).transpose(1, 0, 2)  # [128, 4, 512]
        )

    mu_t = blk(mu)
    var_t = blk(var)
    mu_b = np.ascontiguousarray(mu[:, R_IN]).reshape(1, C)
    var_b = np.ascontiguousarray(var[:, R_IN]).reshape(1, C)
    id32 = np.eye(M_SH, dtype=np.float32)

    # E per-sample transpose + block: [m, p, k, c], r = 128k + p (fp16)
    E16 = np.asarray(E, dtype=np.float16)
    et = np.ascontiguousarray(
        E16.transpose(0, 2, 1)[:, :R_IN, :]             # [256, 512(r), 512(c)]
        .reshape(M_TOTAL, KCH, 128, C)
        .transpose(0, 2, 1, 3)                          # [256, 128, 4, 512]
    )
    eb = np.ascontiguousarray(E[:, :, R_IN], dtype=np.float32)  # [256, 512]

    # x transposed-blocked per core: [p, k, m_local]
    in_maps = []
    for core in range(N_CORES):
        sl = slice(core * M_SH, (core + 1) * M_SH)
        xs = np.asarray(x[sl], dtype=np.float16)        # [32, 512]
        xt = np.ascontiguousarray(
            xs.T.reshape(KCH, 128, M_SH).transpose(1, 0, 2)  # [128, 4, 32]
        )
        in_maps.append({
            "et": np.ascontiguousarray(et[sl]),
            "eb": np.ascontiguousarray(eb[sl]),
            "xt": xt,
            "mu_t": mu_t,
            "var_t": var_t,
            "mu_b": mu_b,
            "var_b": var_b,
            "id32": id32,
        })
    return in_maps


def kernel(x, mu, var, E, shape=None, _trace=False, **_ignored):
    global _COMPILED
    if _COMPILED is None:
        _COMPILED = _build_program()
    nc = _COMPILED
    in_maps = _prep_inputs(np.asarray(x), np.asarray(mu), np.asarray(var), np.asarray(E))
    res = run_bass_kernel_spmd(
        nc, in_maps, core_ids=list(range(N_CORES)), trace=_trace,
    )
    out = np.concatenate([res.results[i]["out"] for i in range(N_CORES)], axis=0)
    if _trace:
        kernel._last_results = res
    return out


# revision 18
# speedup vs baseline: 2.1842x; 1.3816x over previous
"""BNNLinear sampling kernel for Trainium2, data-parallel over 8 NeuronCores.

Computes h[m,c] = sum_r x_ext[m,r] * (mu[c,r] + sqrt(var[c,r]) * E[m,c,r])
with x_ext = concat([x, ones], axis=1), for
  x  [256, 512] f32, mu/var [512, 513] f32, E [256, 512, 513] f32.

Strategy (memory-bound; E must stream through HBM once per pass):
 - Shard the sample axis m across the 8 cores (32 samples each).
 - Host-side layout + fp16 downcast: per-sample transpose of E to [r, c]
   blocked as [m, p, k, c] (r = 128k + p) so each per-sample DMA is one
   contiguous 512 KB fp16 transfer landing as SBUF tile [128p, 4k, 512c];
   mu/var/x are pre-transposed the same way (tiny). fp16 halves the HBM
   stream against the ~358 GB/s per-core HBM roofline; the added rel-err
   (~sqrt(513)*2^-11 of the term scale) is ~100x below the 2e-2 gate.
   All arithmetic (sqrt, multiplies, reductions) is on-chip; accumulation
   stays f32 in PSUM.
 - E stream split across BOTH HWDGE queues (SP: even samples + sample 31
   for ring balance against the ~1.1 MB of constants on Act; Act: odd
   samples + constants): one HWDGE queue tops out ~286 GB/s, two sustain
   ~380 GB/s. E-tile DMAs are issued LOOKAHEAD_SAMPLES ahead of the
   compute loop so a drain copy waiting on PE can never stall the next
   dma_start on the same sequencer long enough to drain the ring.
 - Per sample: one DVE tensor_tensor B = E_t * sqrt(var)_t ([128, 2048]
   fp16 in/out -> 2x perf mode, ~1.1 us), then 4 fp16 PE matmuls
   (stationary = x column chunk [128, 1]) accumulate sum_r over the 4
   r-chunks into a private [1, 512] PSUM row (f32).
 - The mean/bias term hbs[m,c] = x@mu^T + mu_bias + sqrt(var_bias)*E_bias
   is computed once at setup ([32, 512]), downcast to fp16 and pre-written
   to the DRAM output shard. Each finished PSUM row (the E-term only) is
   drained by an Act copy into fp16 [1, 4, C] staging blocks, and the
   per-group output DMA ACCUMULATES (SWDGE CCE add) the staging block onto
   the pre-written mean term. This removes the per-sample PSUM-seeding
   matmul (PE is co-critical with DMA) and keeps every data-dependent
   output DMA on the otherwise-idle Pool/SWDGE ring, where its sem wait
   cannot head-of-line block the E stream. The host casts the fp16 output
   shard back to f32.
"""

import numpy as np
from contextlib import ExitStack

import concourse.bacc as bacc
import concourse.mybir as mybir
import concourse.tile as tile
from concourse.bass_utils import run_bass_kernel_spmd

F32 = mybir.dt.float32
F16 = mybir.dt.float16

N_CORES = 8
M_TOTAL = 256
M_SH = M_TOTAL // N_CORES  # 32 samples per core
C = 512
R_IN = 512                 # r chunks: 4 x 128
KCH = 4
LA = 8                     # E-tile DMA issue lookahead (samples)

_COMPILED = None


def _equeue(nc, m):
    # queue balance: SP carries evens + sample 31, Act carries odds + the
    # ~1.1 MB of constants -> ~8.9 MB per HWDGE ring
    return nc.sync if (m % 2 == 0 or m == M_SH - 1) else nc.scalar


def _build_program(repeat=1):
    nc = bacc.Bacc("TRN2", target_bir_lowering=False, debug=False)

    et_d = nc.dram_tensor("et", [M_SH, 128, KCH, C], F16, kind="ExternalInput").ap()
    eb_d = nc.dram_tensor("eb", [M_SH, C], F32, kind="ExternalInput").ap()
    xt_d = nc.dram_tensor("xt", [128, KCH, M_SH], F16, kind="ExternalInput").ap()
    mu_d = nc.dram_tensor("mu_t", [128, KCH, C], F16, kind="ExternalInput").ap()
    mub_d = nc.dram_tensor("mu_b", [1, C], F32, kind="ExternalInput").ap()
    var_d = nc.dram_tensor("var_t", [128, KCH, C], F16, kind="ExternalInput").ap()
    varb_d = nc.dram_tensor("var_b", [1, C], F32, kind="ExternalInput").ap()
    out_d = nc.dram_tensor("out", [M_SH, C], F16, kind="ExternalOutput").ap()

    total = repeat * M_SH

    with tile.TileContext(nc) as tc, ExitStack() as ctx:
        const = ctx.enter_context(tc.tile_pool(name="const", bufs=1))
        work = ctx.enter_context(tc.tile_pool(name="work", bufs=LA + 2))
        bpool = ctx.enter_context(tc.tile_pool(name="bpool", bufs=8))
        spool = ctx.enter_context(tc.tile_pool(name="spool", bufs=4))
        psum = ctx.enter_context(tc.tile_pool(name="psum", bufs=6, space="PSUM"))
        psum1 = ctx.enter_context(tc.tile_pool(name="psum1", bufs=1, space="PSUM"))

        tiles = {}

        def fetch(i):
            m = i % M_SH
            e_t = work.tile([128, KCH, C], F16, tag="et")
            if m == M_SH - 1:
                # last sample of the round arrives as 4 per-chunk DMAs so
                # its (chunked) multiply and matmuls can start while the
                # stream tail is still in flight
                for k in range(KCH):
                    _equeue(nc, m).dma_start(e_t[:, k, :], et_d[m, :, k, :])
            else:
                _equeue(nc, m).dma_start(e_t[:], et_d[m])
            tiles[i] = e_t

        # ---- E stream: even-sample loads first so the SP ring starts the
        # bulk stream at t~0 (the Act ring starts with the constants).
        for i in range(0, min(LA, total), 2):
            fetch(i)

        # ---- constants, all on the Act HWDGE queue (var first: sqrt path;
        # chunked so each sqrt chunk -- and the chunked first TTs -- can
        # start as soon as its quarter lands)
        var_sb = const.tile([128, KCH, C], F16)
        for k in range(KCH):
            nc.scalar.dma_start(var_sb[:, k, :], var_d[:, k, :])
        xt_sb = const.tile([128, KCH, M_SH], F16)
        nc.scalar.dma_start(xt_sb[:], xt_d)
        varb_sb = const.tile([1, C], F32)
        nc.scalar.dma_start(varb_sb[:], varb_d)
        mu_sb = const.tile([128, KCH, C], F16)
        nc.scalar.dma_start(mu_sb[:], mu_d)
        mub_sb = const.tile([1, C], F32)
        nc.scalar.dma_start(mub_sb[:], mub_d)
        eb_sb = const.tile([M_SH, C], F32)
        nc.scalar.dma_start(eb_sb[:], eb_d)

        for i in range(1, min(LA, total), 2):
            fetch(i)

        s_sb = const.tile([128, KCH, C], F16)
        for k in range(KCH):
            nc.scalar.sqrt(s_sb[:, k, :], var_sb[:, k, :])
        sb_sb = const.tile([1, C], F32)
        nc.scalar.sqrt(sb_sb[:], varb_sb[:])

        ones32 = const.tile([1, M_SH], F32)
        nc.vector.memset(ones32[:], 1.0)
        one16 = const.tile([1, 1], F16)
        nc.vector.memset(one16[:], 1.0)

        # broadcast sqrt(var) bias row to 32 partitions via PE outer product
        ps_b = psum1.tile([M_SH, C], F32)
        nc.tensor.matmul(ps_b[:], lhsT=ones32[:], rhs=sb_sb[:], start=True, stop=True)
        sbb_sb = const.tile([M_SH, C], F32)
        nc.scalar.copy(sbb_sb[:], ps_b[:])

        # h1 = x_t @ mu_t + mu bias row  -> [32, 512] psum, rows = samples
        h1_ps = psum1.tile([M_SH, C], F32)
        for k in range(KCH):
            nc.tensor.matmul(
                h1_ps[:],
                lhsT=xt_sb[:, k, :],
                rhs=mu_sb[:, k, :],
                start=(k == 0), stop=False,
            )
        nc.tensor.matmul(h1_ps[:], lhsT=ones32[:], rhs=mub_sb[:], start=False, stop=True)

        # hbs[m, c] = h1[m, c] + Eb[m, c] * sqrt(var)[c, 512], downcast to
        # fp16 and pre-written to the DRAM output shard; the per-group
        # output DMAs accumulate the PSUM-drained E-term on top (CCE add).
        ebs_sb = const.tile([M_SH, C], F32)
        nc.vector.tensor_tensor(
            out=ebs_sb[:], in0=eb_sb[:], in1=sbb_sb[:], op=mybir.AluOpType.mult
        )
        hbs_sb = const.tile([M_SH, C], F16)
        nc.vector.tensor_tensor(
            out=hbs_sb[:], in0=h1_ps[:], in1=ebs_sb[:], op=mybir.AluOpType.add
        )
        nc.gpsimd.dma_start(out_d[:, :], hbs_sb[:])
        # partition-0 copies of the last 4 hbs rows (PE moving operands
        # must sit at base partition 0; engine APs can't start at partition
        # 28, but DMA descriptors can) for the tail PSUM seeding
        hbs4 = const.tile([1, 4, C], F16)
        nc.gpsimd.dma_start(hbs4[:, :, :], hbs_sb[M_SH - 4 : M_SH, :])

        # ---- main loop over samples ----
        for i in range(total):
            m = i % M_SH
            if i + LA < total:
                fetch(i + LA)
            e_t = tiles.pop(i)
            bt = bpool.tile([128, KCH, C], F16, tag="bt")
            if m >= M_SH - 2:
                # ramp/drain edges: chunk the multiply so each matmul can
                # start as soon as its r-chunk is scaled (and, at the ramp,
                # as soon as that chunk of sqrt(var) is ready) instead of
                # trailing the full-tile op
                for k in range(KCH):
                    nc.vector.tensor_tensor(
                        out=bt[:, k, :], in0=e_t[:, k, :], in1=s_sb[:, k, :],
                        op=mybir.AluOpType.mult,
                    )
            else:
                nc.vector.tensor_tensor(
                    out=bt[:], in0=e_t[:], in1=s_sb[:], op=mybir.AluOpType.mult
                )
            pm = psum.tile([1, C], F32, tag="pm")
            # last group of the last round: seed the PSUM row with hbs[m]
            # via a 1-partition preload matmul (PE is idle in the drain
            # tail) so the final output DMA can be a plain HWDGE store on
            # the empty SP ring instead of a slower SWDGE accumulate
            seed = (i >= total - 4)
            if seed:
                nc.tensor.matmul(
                    pm[:], lhsT=one16[:], rhs=hbs4[:, m - (M_SH - 4), :],
                    start=True, stop=False, skip_group_check=True,
                )
            for k in range(KCH):
                nc.tensor.matmul(
                    pm[:],
                    lhsT=xt_sb[:, k, m : m + 1],
                    rhs=bt[:, k, :],
                    start=(k == 0) and not seed,
                    stop=(k == KCH - 1),
                    skip_group_check=True,
                )
            # drain the E-term: Act copy (engine APs must start at
            # partition 0; DMA can't read PSUM) into fp16 [1, 4, C]
            # staging blocks, accumulated onto the DRAM output shard 4
            # rows at a time on the Pool/SWDGE ring (its sem wait must not
            # sit on an E-carrying HWDGE ring).
            if m % 4 == 0:
                st = spool.tile([1, 4, C], F16, tag="st")
            nc.scalar.copy(st[:, m % 4, :], pm[:])
            if m % 4 == 3:
                if i == total - 1:
                    # seeded rows already hold the full h: plain store
                    nc.sync.dma_start(out_d[m - 3 : m + 1, :], st[:, :, :])
                else:
                    nc.gpsimd.dma_start(
                        out_d[m - 3 : m + 1, :], st[:, :, :],
                        accum_op=mybir.AluOpType.add,
                    )

    nc.compile()
    return nc


def _prep_inputs(x, mu, var, E):
    x = np.asarray(x, dtype=np.float32)
    mu = np.asarray(mu, dtype=np.float32)
    var = np.asarray(var, dtype=np.float32)
    E = np.asarray(E)

    # mu/var transposed-blocked: [p, k, c] with r = 128k + p (r < 512)
    def blk(t):
        tt = np.ascontiguousarray(t.T[:R_IN], dtype=np.float16)  # [512, 512] (r, c)
        return np.ascontiguousarray(
            tt.reshape(KCH, 128, C).transpose(1, 0, 2)  # [128, 4, 512]
        )

    mu_t = blk(mu)
    var_t = blk(var)
    mu_b = np.ascontiguousarray(mu[:, R_IN]).reshape(1, C)
    var_b = np.ascontiguousarray(var[:, R_IN]).reshape(1, C)

    # E per-sample transpose + block: [m, p, k, c], r = 128k + p (fp16)
    E16 = np.asarray(E, dtype=np.float16)
    et = np.ascontiguousarray(
        E16.transpose(0, 2, 1)[:, :R_IN, :]             # [256, 512(r), 512(c)]
        .reshape(M_TOTAL, KCH, 128, C)
        .transpose(0, 2, 1, 3)                          # [256, 128, 4, 512]
    )
    eb = np.ascontiguousarray(E[:, :, R_IN], dtype=np.float32)  # [256, 512]

    # x transposed-blocked per core: [p, k, m_local]
    in_maps = []
    for core in range(N_CORES):
        sl = slice(core * M_SH, (core + 1) * M_SH)
        xs = np.asarray(x[sl], dtype=np.float16)        # [32, 512]
        xt = np.ascontiguousarray(
            xs.T.reshape(KCH, 128, M_SH).transpose(1, 0, 2)  # [128, 4, 32]
        )
        in_maps.append({
            "et": np.ascontiguousarray(et[sl]),
            "eb": np.ascontiguousarray(eb[sl]),
            "xt": xt,
            "mu_t": mu_t,
            "var_t": var_t,
            "mu_b": mu_b,
            "var_b": var_b,
        })
    return in_maps


def kernel(x, mu, var, E, shape=None, _trace=False, **_ignored):
    global _COMPILED
    if _COMPILED is None:
        _COMPILED = _build_program()
    nc = _COMPILED
    in_maps = _prep_inputs(np.asarray(x), np.asarray(mu), np.asarray(var), np.asarray(E))
    res = run_bass_kernel_spmd(
        nc, in_maps, core_ids=list(range(N_CORES)), trace=_trace,
    )
    out = np.concatenate(
        [res.results[i]["out"].astype(np.float32) for i in range(N_CORES)], axis=0
    )
    if _trace:
        kernel._last_results = res
    return out
